# revision 1
# baseline (speedup 1.0000x reference)
"""BinaryTreeLSTM over a complete 18-level binary tree, on 8 Trainium2 cores.

Sharding: contiguous block-sharding of every level across the 8 cores makes
each core own an independent sub-forest (parent p's children stay in the same
core's chunk), so device levels run with zero inter-core communication.
Levels DEV_MIN-1..0 (16383 nodes, 6.2%) are finished on the host from the
device's level-DEV_MIN h/c halves.

Device layout: feature-major bf16 tiles [dims, nodes]; within each core,
every level's nodes are stored in bit-reversed order, so the even(left)/
odd(right) child split between levels is two contiguous column halves.

Keep-state layout: for each parent level, two [128, parent_width] tiles:
  keep_h rows 0:64 = h[0:64] of left children, rows 64:128 = right children
  keep_c rows 0:64 = c[0:64] of left children, rows 64:128 = right children
keep_h IS the (stacked) h_prev operand of one K=128 matmul per gate, and
keep_c IS c_prev verbatim -- no per-level gather copies at all.

Tile schedule: each level's T-wide tiles are processed as pairs
(j, ntiles/2 + j): the pair completes BOTH child-halves of one parent
column block, so the next level can start as soon as the matching producer
pair retires (fine-grained cross-level pipelining), and the pair shares
each PE stationary weight between its two matmuls (halves weight-load
stalls). xT/out_hT columns are laid out in processing order (the host owns
the permutation), keeping all DMA chunk management monotonic.

The LSTM bias rides a constant ones feature row in x; the g-gate tanh is
computed as 2*sigmoid(2g)-1 with host-doubled g weights so one ACT
instruction covers all four gates.
"""

import numpy as np

import concourse.bacc as bacc
import concourse.mybir as mybir
from concourse.tile import TileContext
from concourse.bass_utils import run_bass_kernel_spmd

INPUT = 64
H = 128
HH = H // 2
LEVELS = 18
N_CORES = 8
T = 512           # node-tile width (one fp32 PSUM bank)
DEV_MIN = 14      # lowest tree level computed on device; host does DEV_MIN-1..0
XCHUNK = 8192     # x prefetch chunk (cols)
OCHUNK = 2048     # h output staging chunk (cols)

F32 = mybir.dt.float32
BF16 = mybir.dt.bfloat16

# weight column order (host pre-permutes gate blocks to [i, f, o, g])
GI, GF, GO, GG = 0, 1, 2, 3


def _layout(L=LEVELS):
    """Per-core column layout: leaves first, level DEV_MIN last."""
    levels = list(range(L - 1, DEV_MIN - 1, -1))
    widths = {l: 2 ** (l - 3) for l in levels}
    off = {}
    cur = 0
    for l in levels:
        off[l] = cur
        cur += widths[l]
    return levels, widths, off, cur


def _bitrev_perm(n):
    bits = max(n.bit_length() - 1, 0)
    j = np.arange(n)
    r = np.zeros(n, dtype=np.int64)
    for b in range(bits):
        r |= ((j >> b) & 1) << (bits - 1 - b)
    return r


def _tile_order(ntiles):
    """Pairs (j, mid+j): each pair finishes one parent column block."""
    if ntiles == 1:
        return [(0,)]
    mid = ntiles // 2
    return [(j, mid + j) for j in range(mid)]


def _pos_perm(n):
    """Level-local physical col -> processing-order position mapping.

    Returns idx such that processing position p holds physical col idx[p].
    """
    ntiles = (n + T - 1) // T
    idx = []
    for grp in _tile_order(ntiles):
        for t in grp:
            nt = min(T, n - t * T)
            idx.append(np.arange(t * T, t * T + nt))
    return np.concatenate(idx)


def _x_chunks(levels, off, widths, NPC):
    """Contiguous x DMA chunks: small first chunks so compute starts early,
    then XCHUNK-sized, with the small tail levels merged."""
    chunks = [(0, 1024), (1024, 3072)]
    cur = 4096
    for l in levels:
        if widths[l] >= 2048:
            end = off[l] + widths[l]
            while cur < end:
                step = min(XCHUNK, end - cur)
                chunks.append((cur, step))
                cur += step
        else:
            chunks.append((cur, NPC - cur))
            break
    return chunks


def build_program(L=LEVELS, repeats=1):
    """Build the per-core SPMD Bass program (identical on all cores).

    repeats>1 re-runs the whole level sweep back to back (same outputs) --
    used only for marginal-cost timing, never for the graded path.
    """
    nc = bacc.Bacc("TRN2", target_bir_lowering=False, num_devices=N_CORES)
    levels, widths, off, NPC = _layout(L)
    n_leaf = widths[levels[0]]
    w_last = widths[levels[-1]]  # level DEV_MIN width per core

    xT = nc.dram_tensor("xT", [INPUT + 1, NPC], BF16, kind="ExternalInput").ap()
    wxb = nc.dram_tensor("wxb", [INPUT + 1, 4 * H], BF16,
                         kind="ExternalInput").ap()
    whlr = nc.dram_tensor("whlr", [H, 4 * H], BF16, kind="ExternalInput").ap()
    out_hT = nc.dram_tensor("out_hT", [H, NPC], BF16, kind="ExternalOutput").ap()
    out_kh = nc.dram_tensor("out_kh", [H, w_last // 2], BF16,
                            kind="ExternalOutput").ap()
    out_kc = nc.dram_tensor("out_kc", [H, w_last // 2], BF16,
                            kind="ExternalOutput").ap()

    xchunks = _x_chunks(levels, off, widths, NPC)

    with TileContext(nc) as tc:
        with tc.tile_pool(name="consts", bufs=1) as consts, \
             tc.tile_pool(name="keep", bufs=1) as keep, \
             tc.tile_pool(name="work", bufs=3) as work, \
             tc.tile_pool(name="xin", bufs=3) as xin, \
             tc.tile_pool(name="hout", bufs=3) as hout, \
             tc.tile_pool(name="psum", bufs=2, space="PSUM") as psum:

            wxb_s = consts.tile([INPUT + 1, 4 * H], BF16, name="wxb_s")
            nc.sync.dma_start(out=wxb_s, in_=wxb)
            whlr_s = consts.tile([H, 4 * H], BF16, name="whlr_s")
            nc.sync.dma_start(out=whlr_s, in_=whlr)

            # keep-state ping-pong, sized for the two largest parent levels
            khA = keep.tile([H, n_leaf // 2], BF16, name="khA")
            kcA = keep.tile([H, n_leaf // 2], BF16, name="kcA")
            khB = keep.tile([H, max(n_leaf // 4, 1)], BF16, name="khB")
            kcB = keep.tile([H, max(n_leaf // 4, 1)], BF16, name="kcB")

            def keep_bufs(l):
                """Tiles level l's stash writes (arranged for parent l-1)."""
                return (khA, kcA) if (levels[0] - l) % 2 == 0 else (khB, kcB)

            for _rep in range(repeats):
                xi = -1          # current x chunk index
                xt_ch = None     # current x chunk tile
                hst = None       # current h staging tile
                hst_base = hst_end = 0

                for l in levels:
                    n = widths[l]
                    leaf = l == levels[0]
                    half = n // 2
                    kh_t, kc_t = keep_bufs(l)
                    if leaf:
                        kh_p = kc_p = None
                    else:
                        kh_p, kc_p = keep_bufs(l + 1)
                    ntiles = (n + T - 1) // T

                    def resolve_x(cols, nt):
                        nonlocal xi, xt_ch
                        if xi < 0 or cols >= xchunks[xi][0] + xchunks[xi][1]:
                            xi += 1
                            cb, cw = xchunks[xi]
                            xt_ch = xin.tile([INPUT + 1, XCHUNK], BF16,
                                             tag="xt", name="xt")
                            nc.sync.dma_start(out=xt_ch[:, :cw],
                                              in_=xT[:, cb:cb + cw])
                        xb = cols - xchunks[xi][0]
                        return xt_ch[:, xb:xb + nt]

                    def resolve_hst(cols):
                        nonlocal hst, hst_base, hst_end
                        if hst is None or cols >= hst_end:
                            if hst is not None:
                                nc.sync.dma_start(
                                    out=out_hT[:, hst_base:hst_end],
                                    in_=hst[:, :hst_end - hst_base])
                            hst_base = cols
                            hst_end = min(cols + OCHUNK, NPC)
                            hst = hout.tile([H, OCHUNK], BF16, tag="hst",
                                            name="hst")
                        return hst, cols - hst_base

                    def emit_mms(pts, xts, kcs, nts):
                        """Gate matmuls for 1-2 tiles, stationaries paired."""
                        banks = (((0, GI), (1, GO), (2, GG)) if leaf else
                                 ((0, GI), (1, GF), (2, GO), (3, GG)))
                        for bank, g in banks:
                            for pt, xt, nt in zip(pts, xts, nts):
                                nc.tensor.matmul(
                                    pt[:, bank * T:bank * T + nt],
                                    wxb_s[:, g * H:(g + 1) * H],
                                    xt, start=True, stop=leaf)
                            if not leaf:
                                for pt, kc0, nt in zip(pts, kcs, nts):
                                    nc.tensor.matmul(
                                        pt[:, bank * T:bank * T + nt],
                                        whlr_s[:, g * H:(g + 1) * H],
                                        kh_p[:, kc0:kc0 + nt],
                                        start=False, stop=True)

                    def emit_body(pt, nt, kc0, hstv, ho):
                        """Activations + cell/h + stash for one tile."""
                        ng = 3 if leaf else 4
                        S = work.tile([H, 4 * T], BF16, tag="S", bufs=4,
                                      name="S")
                        nc.scalar.activation(
                            out=S[:, 0:ng * T].rearrange(
                                "p (g n) -> p g n", g=ng)[:, :, :nt],
                            in_=pt[:, 0:ng * T].rearrange(
                                "p (g n) -> p g n", g=ng)[:, :, :nt],
                            func=mybir.ActivationFunctionType.Sigmoid)
                        if leaf:
                            si = S[:, 0:nt]
                            so = S[:, T:T + nt]
                            tg = S[:, 2 * T:2 * T + nt]
                        else:
                            si = S[:, 0:nt]
                            sf = S[:, T:T + nt]
                            so = S[:, 2 * T:2 * T + nt]
                            tg = S[:, 3 * T:3 * T + nt]
                        # host doubled g weights: tanh(g) = 2*sigmoid(2g)-1,
                        # fused into the si-product as (sg*2 - 1)*si
                        acc = work.tile([H, 1], F32, tag="acc", bufs=2,
                                        name="acc")
                        c = work.tile([H, T], BF16, tag="c", name="c")
                        if leaf:
                            nc.vector.affine_mul_reduce(
                                c[:, :nt], acc, tg, si, 2.0, -1.0)
                        else:
                            t1 = work.tile([H, T], BF16, tag="t1", name="t1")
                            nc.vector.affine_mul_reduce(
                                t1[:, :nt], acc, tg, si, 2.0, -1.0)
                            t2 = work.tile([H, T], BF16, tag="t2", name="t2")
                            # Pool is slow (~1.1us/tile); keep it off the
                            # latency-critical narrow tail levels
                            t2_eng = nc.gpsimd if n >= 2048 else nc.vector
                            t2_eng.tensor_mul(t2[:, :nt], sf,
                                              kc_p[:, kc0:kc0 + nt])
                            nc.vector.tensor_add(c[:, :nt], t1[:, :nt],
                                                 t2[:, :nt])

                        tch = work.tile([H, T], BF16, tag="tch", name="tch")
                        nc.scalar.activation(
                            out=tch[:, :nt], in_=c[:, :nt],
                            func=mybir.ActivationFunctionType.Tanh)
                        nc.vector.tensor_mul(hstv[:, ho:ho + nt], so,
                                             tch[:, :nt])

                        # stash h/c halves for the parent level:
                        # left children -> rows 0:64, right -> rows 64:128
                        a, b2 = kc0, kc0 + nt
                        segs = []
                        if a < half:
                            e = min(b2, half)
                            segs.append((0, a, a, e - a))
                        if b2 > half:
                            s0 = max(a, half)
                            segs.append((HH, s0 - half, s0, b2 - s0))
                        for r0, pc, sc, w in segs:
                            nc.vector.tensor_copy(
                                out=kh_t[r0:r0 + HH, pc:pc + w],
                                in_=hstv[0:HH, ho + sc - kc0:
                                         ho + sc - kc0 + w])
                            nc.vector.tensor_copy(
                                out=kc_t[r0:r0 + HH, pc:pc + w],
                                in_=c[0:HH, sc - kc0:sc - kc0 + w])

                    pos = 0
                    for grp in _tile_order(ntiles):
                        metas = []   # (nt, xcols, kc0)
                        for t in grp:
                            nt = min(T, n - t * T)
                            metas.append((nt, off[l] + pos * T, t * T))
                            pos += 1
                        xts = [resolve_x(cols, nt) for nt, cols, _ in metas]
                        hs = [resolve_hst(cols) for _, cols, _ in metas]
                        pts = [psum.tile([H, 4 * T], F32, tag="pt",
                                         name="pt") for _ in grp]
                        emit_mms(pts, xts, [m[2] for m in metas],
                                 [m[0] for m in metas])
                        for j in range(len(grp)):
                            nt, cols, kc0 = metas[j]
                            emit_body(pts[j], nt, kc0, hs[j][0], hs[j][1])

                # flush the last staging chunk
                nc.sync.dma_start(out=out_hT[:, hst_base:hst_end],
                                  in_=hst[:, :hst_end - hst_base])
                # dump level-DEV_MIN h/c halves (parent-arranged) for the host
                kh_t, kc_t = keep_bufs(levels[-1])
                nc.sync.dma_start(out=out_kh, in_=kh_t[:, 0:w_last // 2])
                nc.sync.dma_start(out=out_kc, in_=kc_t[:, 0:w_last // 2])

    nc.compile()
    return nc


_PROGRAMS = {}


def _get_program(L=LEVELS):
    if L not in _PROGRAMS:
        _PROGRAMS[L] = build_program(L)
    return _PROGRAMS[L]


def _prep_weights(W_ih, W_hh, b_ih, b_hh):
    import ml_dtypes
    b = (b_ih + b_hh).astype(np.float32)

    # permute gate blocks from [i, f, g, o] (torch order) to [i, f, o, g]
    def gperm(m):
        return np.concatenate(
            [m[0:H], m[H:2 * H], m[3 * H:4 * H], m[2 * H:3 * H]], axis=0)

    Wx = gperm(W_ih).copy()              # [512, 64]
    Wh = gperm(W_hh).copy()              # [512, 128]
    bp = gperm(b[:, None])[:, 0].copy()  # [512]
    # tanh(g) computed as 2*sigmoid(2g)-1 on device: double g's weights
    Wx[3 * H:4 * H] *= 2.0
    Wh[3 * H:4 * H] *= 2.0
    bp[3 * H:4 * H] *= 2.0

    wxb = np.concatenate([Wx.T, bp[None, :]], axis=0)       # [65, 512]
    # rows 0:64 apply to left-child h, rows 64:128 to right-child h
    whlr = np.concatenate([Wh[:, :HH].T, Wh[:, HH:].T], axis=0)  # [128, 512]
    return (wxb.astype(ml_dtypes.bfloat16),
            whlr.astype(ml_dtypes.bfloat16))


def _col_perms(levels, widths):
    """Per level: global-chunk index for each xT/out_hT column position."""
    perms = {}
    for l in levels:
        n = widths[l]
        perms[l] = _bitrev_perm(n)[_pos_perm(n)]
    return perms


def _make_in_maps(x, W_ih, W_hh, b_ih, b_hh, L=LEVELS):
    import ml_dtypes
    levels, widths, off, NPC = _layout(L)
    wxb, whlr = _prep_weights(W_ih, W_hh, b_ih, b_hh)
    perms = _col_perms(levels, widths)

    in_maps = []
    for k in range(N_CORES):
        xTk = np.empty((INPUT + 1, NPC), ml_dtypes.bfloat16)
        xTk[INPUT, :] = 1.0
        for l in levels:
            n = widths[l]
            start = 2 ** l - 1
            chunk = x[start + k * n: start + (k + 1) * n]  # [n, 64]
            xTk[:INPUT, off[l]:off[l] + n] = chunk[perms[l]].T
        in_maps.append({"xT": xTk, "wxb": wxb, "whlr": whlr})
    return in_maps, perms


def _assemble(results, x, W_ih, W_hh, b_ih, b_hh, perms, L=LEVELS):
    levels, widths, off, NPC = _layout(L)
    n_nodes = 2 ** L - 1
    out = np.zeros((n_nodes, H), np.float32)

    w_last = widths[levels[-1]]           # per-core level-DEV_MIN width
    n_last = w_last * N_CORES             # global level-DEV_MIN count
    h_half = np.zeros((n_last, HH), np.float32)
    c_half = np.zeros((n_last, HH), np.float32)
    bitrev_last = _bitrev_perm(w_last)

    for k in range(N_CORES):
        hk = np.asarray(results[k]["out_hT"]).astype(np.float32).T  # [NPC,128]
        for l in levels:
            n = widths[l]
            start = 2 ** l - 1
            out[start + k * n + perms[l]] = hk[off[l]:off[l] + n]
        kh = np.asarray(results[k]["out_kh"]).astype(np.float32)  # [128, w/2]
        kcv = np.asarray(results[k]["out_kc"]).astype(np.float32)
        # parent-arranged: col j holds left child (rows 0:64) = local col j,
        # right child (rows 64:128) = local col j + w_last//2
        hloc = np.empty((w_last, HH), np.float32)
        cloc = np.empty((w_last, HH), np.float32)
        hw = w_last // 2
        hloc[:hw] = kh[0:HH].T
        hloc[hw:] = kh[HH:H].T
        cloc[:hw] = kcv[0:HH].T
        cloc[hw:] = kcv[HH:H].T
        h_half[k * w_last + bitrev_last] = hloc
        c_half[k * w_last + bitrev_last] = cloc

    # levels DEV_MIN-1 .. 0 on host, mirroring the reference exactly
    b = (b_ih + b_hh).astype(np.float32)

    def sig(v):
        return 1.0 / (1.0 + np.exp(-v))

    hh_prev, cc_prev = h_half, c_half  # halves of the child level, in order
    for lvl in range(DEV_MIN - 1, -1, -1):
        start = 2 ** lvl - 1
        count = 2 ** lvl
        xs = x[start:start + count]
        h_prev = np.concatenate([hh_prev[0::2], hh_prev[1::2]], axis=-1)
        c_prev = np.concatenate([cc_prev[0::2], cc_prev[1::2]], axis=-1)
        gates = xs @ W_ih.T + h_prev @ W_hh.T + b
        gi, gf, gg, go = np.split(gates, 4, axis=-1)
        c = sig(gf) * c_prev + sig(gi) * np.tanh(gg)
        h = sig(go) * np.tanh(c)
        out[start:start + count] = h
        hh_prev, cc_prev = h[:, :HH], c[:, :HH]
    return out


def kernel(x, W_ih, W_hh, b_ih, b_hh):
    x = np.asarray(x, np.float32)
    W_ih = np.asarray(W_ih, np.float32)
    W_hh = np.asarray(W_hh, np.float32)
    b_ih = np.asarray(b_ih, np.float32)
    b_hh = np.asarray(b_hh, np.float32)

    nc = _get_program(LEVELS)
    in_maps, perms = _make_in_maps(x, W_ih, W_hh, b_ih, b_hh, LEVELS)
    res = None
    for attempt in range(3):
        try:
            res = run_bass_kernel_spmd(nc, in_maps,
                                       core_ids=list(range(N_CORES)))
            break
        except Exception:
            # transient device wedge (e.g. NRT_EXEC_UNIT_UNRECOVERABLE);
            # give the runtime a moment and retry
            if attempt == 2:
                raise
            import time as _time
            _time.sleep(10)
    return _assemble(res.results, x, W_ih, W_hh, b_ih, b_hh, perms, LEVELS)

